# revision 1
# baseline (speedup 1.0000x reference)
"""Trainium2 Bass kernel for the NeuralODE problem.

Full inputs -> full output. Internally: data-parallel over 8 NeuronCores
(batch rows 8192 split 1024/core), MLP params replicated.

Math: the reference integrates dy/dt = tanh(y@W1+b1)@W2 + b2 with fixed-dt
Dopri5 (dt0 from the Hairer heuristic on x[0], clamped to the remaining
interval, N_MAX=48 scan slots).  The dt schedule is computed on the host
from the actual inputs, and only steps with dt>0 are materialized on the
device, with all Runge-Kutta stage arithmetic folded into PE matmuls:

  State layout: y^T [D=128 partitions, batch cols], two 512-col blocks/core.
  Per step, per block (N=512):
    Z (PSUM, z-space) accumulates W1^T y_i directly:
      Z  = W1^T y_r                      (1 matmul, f32r)
      stage i=2..6: Z += sum_j dt*(c_ij - c_(i-1)j) * (W2@W1)^T a_j
                                         (15 matmuls, f32r, host-prescaled)
      a_i = tanh(Z + bias_i)             (6 ACT ops, bias host-precomputed)
    K (PSUM, state-space) accumulates the update:
      K  = sum_j dt*b_j * W2^T a_j       (5 matmuls, f32r)
    y_new = (K + bias_y) + y             (1 DVE fused op; y stays exact f32,
                                          it is never rounded through f32r)

  f32r (reduced-precision fp32 matmul mode, 4x faster than fp32 on the PE)
  only ever touches quantities that are scaled by dt (~7e-3), so the
  rounding error lands at ~1e-5 relative per step on y.
"""

import os
import numpy as np

B, D, H = 8192, 128, 128
NCORES = 8
RPC = B // NCORES       # rows per core
NBLK = 2
BN = RPC // NBLK        # 512 cols per block
TIMESCALE = 10.0
N_MAX = 48
DT_SKIP = 1e-7          # steps with dt below this have no observable effect

_A = [
    [1.0 / 5.0],
    [3.0 / 40.0, 9.0 / 40.0],
    [44.0 / 45.0, -56.0 / 15.0, 32.0 / 9.0],
    [19372.0 / 6561.0, -25360.0 / 2187.0, 64448.0 / 6561.0, -212.0 / 729.0],
    [9017.0 / 3168.0, -355.0 / 33.0, 46732.0 / 5247.0, 49.0 / 176.0,
     -5103.0 / 18656.0],
]
_BROW = [35.0 / 384.0, 0.0, 500.0 / 1113.0, 125.0 / 192.0, -2187.0 / 6784.0,
         11.0 / 84.0]
_BJ = [0, 2, 3, 4, 5]          # stages with nonzero b coefficient
_GOFF = [0, 1, 3, 6, 10]       # G-matrix index offset per stage 2..6
NG = 15                        # G matrices per set
NW2 = 5                        # scaled-W2 matrices per set
SETW = (NG + NW2) * 128        # matrix columns per set
NBIAS = 7                      # bias columns per set

_prog_cache = {}
_last_results = None           # test harness introspection


def _f32(a):
    return np.asarray(a, dtype=np.float32)


def _mlp_np(y, W1, b1, W2, b2):
    return _f32(np.tanh(_f32(y @ W1 + b1)) @ W2 + b2)


def _dt0_np(x0, W1, b1, W2, b2):
    """Faithful f32 port of the reference initial_step_size on x[0]."""
    rtol = np.float32(1.4e-8)
    atol = np.float32(1.4e-8)
    y0 = _f32(x0)
    f0 = _mlp_np(y0, W1, b1, W2, b2)
    scale = _f32(atol + np.abs(y0) * rtol)
    d0 = np.float32(np.linalg.norm(_f32(y0 / scale)))
    d1 = np.float32(np.linalg.norm(_f32(f0 / scale)))
    if (d0 < 1e-5) or (d1 < 1e-5):
        h0 = np.float32(1e-6)
    else:
        h0 = np.float32(0.01) * d0 / d1
    y1 = _f32(y0 + h0 * f0)
    f1 = _mlp_np(y1, W1, b1, W2, b2)
    d2 = np.float32(np.linalg.norm(_f32((f1 - f0) / scale))) / h0
    if (d1 <= 1e-15) and (d2 <= 1e-15):
        h1 = np.maximum(np.float32(1e-6), h0 * np.float32(1e-3))
    else:
        h1 = np.float32((np.float32(0.01) / (d1 + d2)) ** (1.0 / 5.0))
    return np.float32(np.minimum(np.float32(100.0) * h0, h1))


def _dt_schedule(T, dt0):
    """Replicates the reference scan's f32 dt sequence."""
    tt = np.float32(0.0)
    dts = []
    for _ in range(N_MAX):
        dt = np.float32(np.clip(T - tt, np.float32(0.0), dt0))
        dts.append(dt)
        tt = np.float32(tt + dt)
    return dts


def _make_bundle(W1, b1, W2, b2, set_dts):
    """Pack per-dt prescaled weight matrices + biases into one f32 array.

    Columns: [W1 | set0 mats | set1 mats | ... | set0 biases | set1 biases...]
    mats per set: 15 G = dt*dc*(W2@W1), then 5 dt*b_j*W2.
    biases per set: 6 ACT stage biases, 1 y-update bias.
    """
    W164 = np.asarray(W1, np.float64)
    W264 = np.asarray(W2, np.float64)
    b164 = np.asarray(b1, np.float64)
    b264 = np.asarray(b2, np.float64)
    P64 = W264 @ W164                      # lhsT for z-space contributions
    W1Tb2 = W164.T @ b264                  # [H]

    deltas = []
    prev = [0.0] * 6
    for row in _A:
        deltas.append([row[j] - prev[j] for j in range(len(row))])
        prev = list(row) + [0.0] * (6 - len(row))

    mats = [_f32(W1)]
    biases = []
    for dt in set_dts:
        dt64 = float(dt)
        for drow in deltas:
            for dc in drow:
                mats.append((dt64 * dc * P64).astype(np.float32))
        for j in _BJ:
            mats.append((dt64 * _BROW[j] * W264).astype(np.float32))
        biases.append(b164.astype(np.float32))          # stage 1
        for row in _A:
            biases.append((b164 + dt64 * sum(row) * W1Tb2).astype(np.float32))
        biases.append((dt64 * sum(_BROW) * b264).astype(np.float32))
    mat = np.concatenate(mats, axis=1)
    bias = np.stack(biases, axis=1)
    return np.concatenate([mat, bias], axis=1).astype(np.float32)


def _build_program(n_sets, step_sets):
    import concourse.bacc as bacc
    import concourse.mybir as mybir
    from concourse.tile import TileContext

    f32 = mybir.dt.float32
    f32r = mybir.dt.float32r
    ADD = mybir.AluOpType.add
    TANH = mybir.ActivationFunctionType.Tanh

    CW_M = 128 + n_sets * SETW
    CW = CW_M + n_sets * NBIAS

    nc = bacc.Bacc("TRN2", target_bir_lowering=False, debug=False,
                   num_devices=NCORES)
    x_in = nc.dram_tensor("xT", [D, RPC], f32, kind="ExternalInput")
    w_in = nc.dram_tensor("wb", [128, CW], f32, kind="ExternalInput")
    y_out = nc.dram_tensor("yT", [D, RPC], f32, kind="ExternalOutput")

    with TileContext(nc) as tc:
        with tc.tile_pool(name="const", bufs=1) as cpool, \
             tc.tile_pool(name="work", bufs=2) as wpool, \
             tc.tile_pool(name="psum", bufs=2, space="PSUM") as ppool:
            wb = cpool.tile([128, CW], f32)
            nc.sync.dma_start(out=wb[:], in_=w_in[:])
            xt = cpool.tile([D, RPC], f32)
            nc.sync.dma_start(out=xt[:], in_=x_in[:])
            wr = cpool.tile([128, CW_M], f32r)
            # cast weights f32->f32r; first chunk covers W1+set0 so step 1
            # can start while later sets still cast
            c1 = min(128 + SETW, CW_M)
            nc.vector.tensor_copy(wr[:, 0:c1], wb[:, 0:c1])
            if c1 < CW_M:
                nc.vector.tensor_copy(wr[:, c1:CW_M], wb[:, c1:CW_M])

            def gmat(s, idx):
                o = 128 + s * SETW + idx * 128
                return wr[:, o:o + 128]

            def w2mat(s, j5):
                o = 128 + s * SETW + (NG + j5) * 128
                return wr[:, o:o + 128]

            def bias(s, i):
                o = CW_M + s * NBIAS + i
                return wb[:, o:o + 1]

            y_cur = [xt[:, b * BN:(b + 1) * BN] for b in range(NBLK)]
            nsteps = len(step_sets)
            for step, sid in enumerate(step_sets):
                y_nxt = [None] * NBLK
                for b in range(NBLK):
                    yr = wpool.tile([D, BN], f32r, tag=f"yr{b}")
                    nc.vector.tensor_copy(yr[:], y_cur[b])
                    Z = ppool.tile([H, BN], f32, tag=f"Z{b}")
                    nc.tensor.matmul(Z[:], wr[:, 0:128], yr[:],
                                     start=True, stop=False,
                                     skip_group_check=True)
                    a = []
                    for i in range(6):
                        if i > 0:
                            for j in range(i):
                                nc.tensor.matmul(
                                    Z[:], gmat(sid, _GOFF[i - 1] + j), a[j][:],
                                    start=False, stop=(i == 5 and j == 4),
                                    skip_group_check=True)
                        ai = wpool.tile([H, BN], f32r, tag=f"a{b}_{i}")
                        nc.scalar.activation(ai[:], Z[:], TANH,
                                             bias=bias(sid, i), scale=1.0)
                        a.append(ai)
                    K = ppool.tile([D, BN], f32, tag=f"K{b}")
                    for n5, j in enumerate(_BJ):
                        nc.tensor.matmul(K[:], w2mat(sid, n5), a[j][:],
                                         start=(n5 == 0), stop=(n5 == NW2 - 1),
                                         skip_group_check=True)
                    yn = wpool.tile([D, BN], f32, tag=f"y{b}")
                    nc.vector.scalar_tensor_tensor(
                        yn[:], K[:], bias(sid, 6), y_cur[b], op0=ADD, op1=ADD)
                    y_nxt[b] = yn[:]
                y_cur = y_nxt
                if step == nsteps - 1:
                    for b in range(NBLK):
                        nc.sync.dma_start(out=y_out[:, b * BN:(b + 1) * BN],
                                          in_=y_cur[b])
    nc.compile()
    return nc


def kernel(t, x, W1, b1, W2, b2):
    global _last_results
    t = _f32(t)
    x = _f32(x)
    W1 = _f32(W1)
    b1 = _f32(b1)
    W2 = _f32(W2)
    b2 = _f32(b2)
    assert x.shape == (B, D)

    dt0 = _dt0_np(x[0], W1, b1, W2, b2)
    T = np.float32(t[0] / np.float32(TIMESCALE))
    dts = [dt for dt in _dt_schedule(T, dt0) if dt > DT_SKIP]
    if not dts:
        return np.stack([x, x]).astype(np.float32)

    set_dts = []
    step_sets = []
    for dt in dts:
        val = float(dt)
        if set_dts and set_dts[-1] == val:
            step_sets.append(len(set_dts) - 1)
        elif val in set_dts:
            step_sets.append(set_dts.index(val))
        else:
            set_dts.append(val)
            step_sets.append(len(set_dts) - 1)

    key = (len(set_dts), tuple(step_sets))
    if key not in _prog_cache:
        _prog_cache[key] = _build_program(len(set_dts), tuple(step_sets))
    nc = _prog_cache[key]

    bundle = _make_bundle(W1, b1, W2, b2, set_dts)
    in_maps = []
    for c in range(NCORES):
        xT_c = np.ascontiguousarray(x[c * RPC:(c + 1) * RPC].T)
        in_maps.append({"xT": xT_c, "wb": bundle})

    from concourse.bass_utils import run_bass_kernel_spmd
    res = run_bass_kernel_spmd(nc, in_maps, list(range(NCORES)))
    _last_results = res

    y = np.empty((B, D), np.float32)
    for c in range(NCORES):
        y[c * RPC:(c + 1) * RPC] = res.results[c]["yT"].T
    return np.stack([x, y]).astype(np.float32)
